# revision 10
# baseline (speedup 1.0000x reference)
"""Trainium2 Bass kernel for nn_DeepwiseAtn (dense_cnn).

Computation (reference):
    scale[b,c] = mean(context[b,c,:,:])
    out[b,o,hw] = sum_c w1[o,c] * (x[b,c,hw] * scale[b,c]) + b1[o]

Key algebraic rewrite: fold the per-(b,c) scale INTO the 1x1-conv weight
instead of scaling x elementwise:
    W_b[c,o] = w1t[c,o] * rowsum(context[b,c,:]) / HW
    out[b]   = W_b.T @ x[b] + b1            (x streams straight into the PE)
This removes an 8 MiB/core elementwise pass over x. The 1/HW mean factor
is folded into w1t on the host.

Sharding: data-parallel over batch B=16 across 8 cores (2 batches/core);
w1/b1 replicated. Per-core HBM traffic: 16 MiB in + 8 MiB out.
"""

from contextlib import ExitStack

import numpy as np

import concourse.bass as bass
import concourse.tile as tile
from concourse import bacc, mybir
from concourse.bass_utils import run_bass_kernel_spmd

B, C, HW, OUT = 16, 256, 64 * 64, 256
N_CORES = 8
B_LOC = B // N_CORES  # 2 batches per core
P = 128               # SBUF partitions
KH = C // P           # 2 contraction halves
MH = OUT // P         # 2 output halves
NCHUNK = 512          # one PSUM bank of fp32
NN = HW // NCHUNK     # 8 free-dim chunks

_cache = {}


def _emit_body(nc, tc, pools, aps):
    """One full kernel iteration (loads + compute + stores)."""
    f32 = mybir.dt.float32
    ctxpool, xpool, smallpool, opool, pspool = pools
    x_d, ctx_d, out_d, w_sb, bias_sb = aps

    # Per (b, c-half): context row-sum -> scale folded into weight copy
    x_sb = [[None] * KH for _ in range(B_LOC)]
    wsc = [[None] * KH for _ in range(B_LOC)]
    for b in range(B_LOC):
        for kh in range(KH):
            ct = ctxpool.tile([P, HW], f32, tag="ctx")
            nc.sync.dma_start(ct[:], ctx_d[b, kh * P : (kh + 1) * P, :])
            sums = smallpool.tile([P, 1], f32, tag=f"sums{b}{kh}")
            nc.vector.tensor_reduce(
                sums[:], ct[:], axis=mybir.AxisListType.X, op=mybir.AluOpType.add
            )
            ws = smallpool.tile([P, OUT], f32, tag=f"wsc{b}{kh}")
            # ws = w1t_pre * rowsum == w1t * mean(context)  (1/HW on host)
            nc.scalar.mul(ws[:], w_sb[kh][:], sums[:])
            wsc[b][kh] = ws
            xt = xpool.tile([P, HW], f32, tag=f"x{b}{kh}")
            nc.sync.dma_start(xt[:], x_d[b, kh * P : (kh + 1) * P, :])
            x_sb[b][kh] = xt

    # Pointwise conv: out[b, o, n] = sum_c W_b[c, o] * x[b, c, n] + b1[o]
    for b in range(B_LOC):
        for mh in range(MH):
            for n in range(NN):
                ps = pspool.tile([P, NCHUNK], f32, tag="ps")
                for kh in range(KH):
                    nc.tensor.matmul(
                        ps[:],
                        lhsT=wsc[b][kh][:, mh * P : (mh + 1) * P],
                        rhs=x_sb[b][kh][:, n * NCHUNK : (n + 1) * NCHUNK],
                        start=(kh == 0),
                        stop=(kh == KH - 1),
                    )
                osb = opool.tile([P, NCHUNK], f32, tag="osb")
                nc.scalar.activation(
                    osb[:],
                    ps[:],
                    mybir.ActivationFunctionType.Identity,
                    bias=bias_sb[mh][:],
                )
                nc.scalar.dma_start(
                    out_d[b, mh * P : (mh + 1) * P, n * NCHUNK : (n + 1) * NCHUNK],
                    osb[:],
                )


def _build(reps=1):
    key = ("nc", reps)
    if key in _cache:
        return _cache[key]

    f32 = mybir.dt.float32
    nc = bacc.Bacc("TRN2", target_bir_lowering=False, debug=False)

    x_d = nc.dram_tensor("x", [B_LOC, C, HW], f32, kind="ExternalInput").ap()
    ctx_d = nc.dram_tensor("ctx", [B_LOC, C, HW], f32, kind="ExternalInput").ap()
    w1t_d = nc.dram_tensor("w1t", [C, OUT], f32, kind="ExternalInput").ap()
    b1_d = nc.dram_tensor("b1r", [MH, P, 1], f32, kind="ExternalInput").ap()
    out_d = nc.dram_tensor("out", [B_LOC, C, HW], f32, kind="ExternalOutput").ap()

    with tile.TileContext(nc) as tc, ExitStack() as st:
        wpool = st.enter_context(tc.tile_pool(name="w", bufs=1))
        ctxpool = st.enter_context(tc.tile_pool(name="ctx", bufs=2))
        xpool = st.enter_context(tc.tile_pool(name="x", bufs=1))
        smallpool = st.enter_context(tc.tile_pool(name="small", bufs=1))
        opool = st.enter_context(tc.tile_pool(name="osb", bufs=8))
        pspool = st.enter_context(tc.tile_pool(name="ps", bufs=8, space="PSUM"))

        # Replicated weights + bias (loaded once, reused across reps)
        w_sb = []
        for kh in range(KH):
            w = wpool.tile([P, OUT], f32, tag=f"w{kh}")
            nc.sync.dma_start(w[:], w1t_d[kh * P : (kh + 1) * P, :])
            w_sb.append(w)
        bias_sb = []
        for mh in range(MH):
            bt = smallpool.tile([P, 1], f32, tag=f"bias{mh}")
            nc.sync.dma_start(bt[:], b1_d[mh])
            bias_sb.append(bt)

        pools = (ctxpool, xpool, smallpool, opool, pspool)
        aps = (x_d, ctx_d, out_d, w_sb, bias_sb)
        if reps == 1:
            _emit_body(nc, tc, pools, aps)
        else:
            # HW loop for timing runs: per-iteration back-edge cost ~2us.
            with tc.For_i(0, reps, 1):
                _emit_body(nc, tc, pools, aps)

    nc.compile()
    _cache[key] = nc
    return nc


def run(inputs, reps=1, trace=False, **trace_kwargs):
    nc = _build(reps)
    x = np.ascontiguousarray(inputs["x"], dtype=np.float32).reshape(B, C, HW)
    ctx = np.ascontiguousarray(inputs["context"], dtype=np.float32).reshape(B, C, HW)
    w1t = np.ascontiguousarray(inputs["w1"].T.astype(np.float32) * (1.0 / HW))
    b1r = np.ascontiguousarray(inputs["b1"], dtype=np.float32).reshape(MH, P, 1)
    in_maps = [
        {
            "x": x[i * B_LOC : (i + 1) * B_LOC],
            "ctx": ctx[i * B_LOC : (i + 1) * B_LOC],
            "w1t": w1t,
            "b1r": b1r,
        }
        for i in range(N_CORES)
    ]
    res = run_bass_kernel_spmd(
        nc, in_maps, list(range(N_CORES)), trace=trace, **trace_kwargs
    )
    out = np.concatenate([r["out"] for r in res.results], axis=0)
    return out.reshape(B, C, 64, 64), res


def kernel(**inputs):
    out, _ = run(inputs)
    return out


# revision 18
# speedup vs baseline: 3.1037x; 3.1037x over previous
"""Trainium2 Bass kernel for nn_DeepwiseAtn (dense_cnn).

Computation (reference):
    scale[b,c] = mean(context[b,c,:,:])
    out[b,o,hw] = sum_c w1[o,c] * (x[b,c,hw] * scale[b,c]) + b1[o]

Key algebraic rewrite: fold the per-(b,c) scale INTO the 1x1-conv weight
instead of scaling x elementwise:
    W_b[c,o] = w1t[c,o] * rowsum(context[b,c,:]) / HW
    out[b]   = W_b.T @ x[b] + b1            (x streams straight into the PE)
This removes an 8 MiB/core elementwise pass over x. The 1/HW mean factor
is folded into w1t on the host.

Sharding: data-parallel over batch B=16 across 8 cores (2 batches/core);
w1/b1 replicated. Per-core HBM traffic: 16 MiB in + 8 MiB out.
"""

from contextlib import ExitStack

import numpy as np

import concourse.bass as bass
import concourse.tile as tile
from concourse import bacc, mybir
from concourse.bass_utils import run_bass_kernel_spmd

B, C, HW, OUT = 16, 256, 64 * 64, 256
N_CORES = 8
B_LOC = B // N_CORES  # 2 batches per core
P = 128               # SBUF partitions
KH = C // P           # 2 contraction halves
MH = OUT // P         # 2 output halves
NCHUNK = 512          # one PSUM bank of fp32
NN = HW // NCHUNK     # 8 free-dim chunks

_cache = {}


def _emit_loads(nc, tc, pools, aps, compute=True):
    """Input loads (+ scale computation when compute=True)."""
    f32 = mybir.dt.float32
    ctxpool, xpool, smallpool, opool, pspool = pools
    x_d, ctx_d, out_d, w_sb, bias_sb = aps

    x_sb = [[None] * KH for _ in range(B_LOC)]
    wsc = [[None] * KH for _ in range(B_LOC)]
    for b in range(B_LOC):
        for kh in range(KH):
            ct = ctxpool.tile([P, HW], f32, tag="ctx")
            nc.sync.dma_start(ct[:], ctx_d[b, kh * P : (kh + 1) * P, :])
            if compute:
                sums = smallpool.tile([P, 1], f32, tag=f"sums{b}{kh}")
                nc.vector.tensor_reduce(
                    sums[:], ct[:], axis=mybir.AxisListType.X, op=mybir.AluOpType.add
                )
                ws = smallpool.tile([P, OUT], f32, tag=f"wsc{b}{kh}")
                # ws = w1t_pre * rowsum == w1t * mean(context)  (1/HW on host)
                nc.scalar.mul(ws[:], w_sb[kh][:], sums[:])
                wsc[b][kh] = ws
            xt = xpool.tile([P, HW], f32, tag=f"x{b}{kh}")
            nc.sync.dma_start(xt[:], x_d[b, kh * P : (kh + 1) * P, :])
            x_sb[b][kh] = xt
    return x_sb, wsc


def _emit_compute(nc, tc, pools, aps, x_sb, wsc, store=True, mm_dt=None):
    """Pointwise conv: out[b,o,n] = sum_c W_b[c,o] * x[b,c,n] + b1[o]."""
    f32 = mybir.dt.float32
    ctxpool, xpool, smallpool, opool, pspool = pools
    x_d, ctx_d, out_d, w_sb, bias_sb = aps

    def cast(ap):
        return ap.bitcast(mm_dt) if mm_dt is not None else ap

    for b in range(B_LOC):
        for mh in range(MH):
            for n in range(NN):
                ps = pspool.tile([P, NCHUNK], f32, tag="ps")
                for kh in range(KH):
                    nc.tensor.matmul(
                        ps[:],
                        lhsT=cast(wsc[b][kh][:, mh * P : (mh + 1) * P]),
                        rhs=cast(x_sb[b][kh][:, n * NCHUNK : (n + 1) * NCHUNK]),
                        start=(kh == 0),
                        stop=(kh == KH - 1),
                    )
                osb = opool.tile([P, NCHUNK], f32, tag="osb")
                nc.scalar.activation(
                    osb[:],
                    ps[:],
                    mybir.ActivationFunctionType.Identity,
                    bias=bias_sb[mh][:],
                )
                if store:
                    nc.scalar.dma_start(
                        out_d[
                            b, mh * P : (mh + 1) * P, n * NCHUNK : (n + 1) * NCHUNK
                        ],
                        osb[:],
                    )


def _emit_stores_only(nc, tc, pools, aps, osb_src):
    _, _, out_d, _, _ = aps
    for b in range(B_LOC):
        for mh in range(MH):
            for n in range(NN):
                nc.scalar.dma_start(
                    out_d[b, mh * P : (mh + 1) * P, n * NCHUNK : (n + 1) * NCHUNK],
                    osb_src[:],
                )


def _emit_body(nc, tc, pools, aps, variant="full"):
    if variant in ("full", "fullr"):
        mm_dt = mybir.dt.float32r if variant == "fullr" else None
        x_sb, wsc = _emit_loads(nc, tc, pools, aps, compute=True)
        _emit_compute(nc, tc, pools, aps, x_sb, wsc, store=True, mm_dt=mm_dt)
    elif variant == "dma":
        _emit_loads(nc, tc, pools, aps, compute=False)
    elif variant == "loads_stores":
        x_sb, _ = _emit_loads(nc, tc, pools, aps, compute=False)
        # store from a couple of the loaded tiles (no compute dependency)
        f32 = mybir.dt.float32
        out_d = aps[2]
        for b in range(B_LOC):
            for mh in range(MH):
                for n in range(NN):
                    nc.scalar.dma_start(
                        out_d[
                            b, mh * P : (mh + 1) * P, n * NCHUNK : (n + 1) * NCHUNK
                        ],
                        x_sb[b][mh][:, n * NCHUNK : (n + 1) * NCHUNK],
                    )
    else:
        raise ValueError(variant)


def _build(reps=1, variant="full"):
    key = ("nc", reps, variant)
    if key in _cache:
        return _cache[key]

    f32 = mybir.dt.float32
    nc = bacc.Bacc("TRN2", target_bir_lowering=False, debug=False)

    x_d = nc.dram_tensor("x", [B_LOC, C, HW], f32, kind="ExternalInput").ap()
    ctx_d = nc.dram_tensor("ctx", [B_LOC, C, HW], f32, kind="ExternalInput").ap()
    w1t_d = nc.dram_tensor("w1t", [C, OUT], f32, kind="ExternalInput").ap()
    b1_d = nc.dram_tensor("b1r", [MH, P, 1], f32, kind="ExternalInput").ap()
    out_d = nc.dram_tensor("out", [B_LOC, C, HW], f32, kind="ExternalOutput").ap()

    with tile.TileContext(nc) as tc, ExitStack() as st:
        wpool = st.enter_context(tc.tile_pool(name="w", bufs=1))
        ctxpool = st.enter_context(tc.tile_pool(name="ctx", bufs=2))
        xpool = st.enter_context(tc.tile_pool(name="x", bufs=1))
        smallpool = st.enter_context(tc.tile_pool(name="small", bufs=1))
        opool = st.enter_context(tc.tile_pool(name="osb", bufs=8))
        pspool = st.enter_context(tc.tile_pool(name="ps", bufs=8, space="PSUM"))

        # Replicated weights + bias (loaded once, reused across reps)
        w_sb = []
        for kh in range(KH):
            w = wpool.tile([P, OUT], f32, tag=f"w{kh}")
            nc.sync.dma_start(w[:], w1t_d[kh * P : (kh + 1) * P, :])
            w_sb.append(w)
        bias_sb = []
        for mh in range(MH):
            bt = smallpool.tile([P, 1], f32, tag=f"bias{mh}")
            nc.sync.dma_start(bt[:], b1_d[mh])
            bias_sb.append(bt)

        pools = (ctxpool, xpool, smallpool, opool, pspool)
        aps = (x_d, ctx_d, out_d, w_sb, bias_sb)
        if variant in ("pe", "per"):
            # compute-only loop: loads hoisted out of the timing loop
            mm_dt = mybir.dt.float32r if variant == "per" else None
            x_sb, wsc = _emit_loads(nc, tc, pools, aps, compute=True)
            if reps == 1:
                _emit_compute(nc, tc, pools, aps, x_sb, wsc, store=False, mm_dt=mm_dt)
            else:
                with tc.For_i(0, reps, 1):
                    _emit_compute(
                        nc, tc, pools, aps, x_sb, wsc, store=False, mm_dt=mm_dt
                    )
        elif reps == 1:
            _emit_body(nc, tc, pools, aps, variant)
        else:
            # HW loop for timing runs: per-iteration back-edge cost ~2us.
            with tc.For_i(0, reps, 1):
                _emit_body(nc, tc, pools, aps, variant)

    nc.compile()
    _cache[key] = nc
    return nc


def run(inputs, reps=1, variant="full", n_cores=N_CORES, trace=False, **trace_kwargs):
    nc = _build(reps, variant)
    x = np.ascontiguousarray(inputs["x"], dtype=np.float32).reshape(B, C, HW)
    ctx = np.ascontiguousarray(inputs["context"], dtype=np.float32).reshape(B, C, HW)
    w1t = np.ascontiguousarray(inputs["w1"].T.astype(np.float32) * (1.0 / HW))
    b1r = np.ascontiguousarray(inputs["b1"], dtype=np.float32).reshape(MH, P, 1)
    in_maps = [
        {
            "x": x[i * B_LOC : (i + 1) * B_LOC],
            "ctx": ctx[i * B_LOC : (i + 1) * B_LOC],
            "w1t": w1t,
            "b1r": b1r,
        }
        for i in range(n_cores)
    ]
    res = run_bass_kernel_spmd(
        nc, in_maps, list(range(n_cores)), trace=trace, **trace_kwargs
    )
    out = np.concatenate([r["out"] for r in res.results], axis=0)
    if n_cores == N_CORES:
        out = out.reshape(B, C, 64, 64)
    return out, res


def kernel(**inputs):
    out, _ = run(inputs)
    return out


# revision 24
# speedup vs baseline: 3.4871x; 1.1236x over previous
"""Trainium2 Bass kernel for nn_DeepwiseAtn (dense_cnn).

Computation (reference):
    scale[b,c] = mean(context[b,c,:,:])
    out[b,o,hw] = sum_c w1[o,c] * (x[b,c,hw] * scale[b,c]) + b1[o]

Key algebraic rewrite: fold the per-(b,c) scale INTO the 1x1-conv weight
instead of scaling x elementwise:
    W_b[c,o] = w1t[c,o] * rowsum(context[b,c,:]) / HW
    out[b]   = W_b.T @ x[b] + b1            (x streams straight into the PE)
This removes an 8 MiB/core elementwise pass over x. The 1/HW mean factor
is folded into w1t on the host.

Sharding: data-parallel over batch B=16 across 8 cores (2 batches/core);
w1/b1 replicated. Per-core HBM traffic: 16 MiB in + 8 MiB out.
"""

from contextlib import ExitStack

import numpy as np

import concourse.bass as bass
import concourse.tile as tile
from concourse import bacc, mybir
from concourse.bass_utils import run_bass_kernel_spmd

B, C, HW, OUT = 16, 256, 64 * 64, 256
N_CORES = 8
B_LOC = B // N_CORES  # 2 batches per core
P = 128               # SBUF partitions
KH = C // P           # 2 contraction halves
MH = OUT // P         # 2 output halves
NCHUNK = 512          # one PSUM bank of fp32
NN = HW // NCHUNK     # 8 free-dim chunks

_cache = {}


def _emit_loads(nc, tc, pools, aps, compute=True, mm_dt=None):
    """Input loads (+ scale computation when compute=True)."""
    f32 = mybir.dt.float32
    xdt = mm_dt if mm_dt is not None else f32
    ctxpool, xpool, smallpool, opool, pspool = pools
    x_d, ctx_d, out_d, w_sb, bias_sb = aps

    x_sb = [[None] * KH for _ in range(B_LOC)]
    wsc = [[None] * KH for _ in range(B_LOC)]
    for b in range(B_LOC):
        for kh in range(KH):
            ct = ctxpool.tile([P, HW], f32, tag="ctx")
            nc.sync.dma_start(ct[:], ctx_d[b, kh * P : (kh + 1) * P, :])
            if compute:
                sums = smallpool.tile([P, 1], f32, tag=f"sums{b}{kh}")
                nc.vector.tensor_reduce(
                    sums[:], ct[:], axis=mybir.AxisListType.X, op=mybir.AluOpType.add
                )
                ws = smallpool.tile([P, OUT], xdt, tag=f"wsc{b}{kh}")
                # ws = w1t_pre * rowsum == w1t * mean(context)  (1/HW on host)
                nc.scalar.mul(ws[:], w_sb[kh][:], sums[:])
                wsc[b][kh] = ws
            xt = xpool.tile([P, HW], xdt, tag=f"x{b}{kh}")
            nc.sync.dma_start(xt[:], x_d[b, kh * P : (kh + 1) * P, :])
            x_sb[b][kh] = xt
    return x_sb, wsc


def _emit_compute(nc, tc, pools, aps, x_sb, wsc, store=True, mm_dt=None):
    """Pointwise conv: out[b,o,n] = sum_c W_b[c,o] * x[b,c,n] + b1[o]."""
    f32 = mybir.dt.float32
    ctxpool, xpool, smallpool, opool, pspool = pools
    x_d, ctx_d, out_d, w_sb, bias_sb = aps

    # kh-sweep ordering: per (b, mh), run all 8 n-chunks' kh=0 matmuls
    # first (needs only x[b,0]), then the kh=1 closers. The PE starts as
    # soon as the FIRST x half-tile lands instead of waiting for both.
    for b in range(B_LOC):
        for mh in range(MH):
            pss = []
            for n in range(NN):
                ps = pspool.tile([P, NCHUNK], f32, tag="ps")
                pss.append(ps)
                nc.tensor.matmul(
                    ps[:],
                    lhsT=wsc[b][0][:, mh * P : (mh + 1) * P],
                    rhs=x_sb[b][0][:, n * NCHUNK : (n + 1) * NCHUNK],
                    start=True,
                    stop=False,
                )
            for n in range(NN):
                ps = pss[n]
                nc.tensor.matmul(
                    ps[:],
                    lhsT=wsc[b][1][:, mh * P : (mh + 1) * P],
                    rhs=x_sb[b][1][:, n * NCHUNK : (n + 1) * NCHUNK],
                    start=False,
                    stop=True,
                )
                osb = opool.tile([P, NCHUNK], f32, tag="osb")
                nc.scalar.activation(
                    osb[:],
                    ps[:],
                    mybir.ActivationFunctionType.Identity,
                    bias=bias_sb[mh][:],
                )
                if store:
                    nc.scalar.dma_start(
                        out_d[
                            b, mh * P : (mh + 1) * P, n * NCHUNK : (n + 1) * NCHUNK
                        ],
                        osb[:],
                    )


def _emit_stores_only(nc, tc, pools, aps, osb_src):
    _, _, out_d, _, _ = aps
    for b in range(B_LOC):
        for mh in range(MH):
            for n in range(NN):
                nc.scalar.dma_start(
                    out_d[b, mh * P : (mh + 1) * P, n * NCHUNK : (n + 1) * NCHUNK],
                    osb_src[:],
                )


def _emit_body(nc, tc, pools, aps, variant="full"):
    if variant in ("full", "fullr"):
        mm_dt = mybir.dt.float32r if variant == "fullr" else None
        x_sb, wsc = _emit_loads(nc, tc, pools, aps, compute=True, mm_dt=mm_dt)
        _emit_compute(nc, tc, pools, aps, x_sb, wsc, store=True)
    elif variant == "dma":
        _emit_loads(nc, tc, pools, aps, compute=False)
    elif variant == "loads_stores":
        x_sb, _ = _emit_loads(nc, tc, pools, aps, compute=False)
        # store from a couple of the loaded tiles (no compute dependency)
        f32 = mybir.dt.float32
        out_d = aps[2]
        for b in range(B_LOC):
            for mh in range(MH):
                for n in range(NN):
                    nc.scalar.dma_start(
                        out_d[
                            b, mh * P : (mh + 1) * P, n * NCHUNK : (n + 1) * NCHUNK
                        ],
                        x_sb[b][mh][:, n * NCHUNK : (n + 1) * NCHUNK],
                    )
    else:
        raise ValueError(variant)


def _build(reps=1, variant="full"):
    key = ("nc", reps, variant)
    if key in _cache:
        return _cache[key]

    f32 = mybir.dt.float32
    xdt = mybir.dt.float32r if variant.endswith("r") else f32
    nc = bacc.Bacc("TRN2", target_bir_lowering=False, debug=False)

    x_d = nc.dram_tensor("x", [B_LOC, C, HW], xdt, kind="ExternalInput").ap()
    ctx_d = nc.dram_tensor("ctx", [B_LOC, C, HW], f32, kind="ExternalInput").ap()
    w1t_d = nc.dram_tensor("w1t", [C, OUT], f32, kind="ExternalInput").ap()
    b1_d = nc.dram_tensor("b1r", [MH, P, 1], f32, kind="ExternalInput").ap()
    out_d = nc.dram_tensor("out", [B_LOC, C, HW], f32, kind="ExternalOutput").ap()

    with tile.TileContext(nc) as tc, ExitStack() as st:
        wpool = st.enter_context(tc.tile_pool(name="w", bufs=1))
        ctxpool = st.enter_context(tc.tile_pool(name="ctx", bufs=2))
        xpool = st.enter_context(tc.tile_pool(name="x", bufs=1))
        smallpool = st.enter_context(tc.tile_pool(name="small", bufs=1))
        opool = st.enter_context(tc.tile_pool(name="osb", bufs=8))
        pspool = st.enter_context(tc.tile_pool(name="ps", bufs=8, space="PSUM"))

        # Replicated weights + bias (loaded once, reused across reps)
        w_sb = []
        for kh in range(KH):
            w = wpool.tile([P, OUT], f32, tag=f"w{kh}")
            nc.sync.dma_start(w[:], w1t_d[kh * P : (kh + 1) * P, :])
            w_sb.append(w)
        bias_sb = []
        for mh in range(MH):
            bt = smallpool.tile([P, 1], f32, tag=f"bias{mh}")
            nc.sync.dma_start(bt[:], b1_d[mh])
            bias_sb.append(bt)

        pools = (ctxpool, xpool, smallpool, opool, pspool)
        aps = (x_d, ctx_d, out_d, w_sb, bias_sb)
        if variant in ("pe", "per"):
            # compute-only loop: loads hoisted out of the timing loop
            mm_dt = mybir.dt.float32r if variant == "per" else None
            x_sb, wsc = _emit_loads(nc, tc, pools, aps, compute=True, mm_dt=mm_dt)
            if reps == 1:
                _emit_compute(nc, tc, pools, aps, x_sb, wsc, store=False)
            else:
                with tc.For_i(0, reps, 1):
                    _emit_compute(nc, tc, pools, aps, x_sb, wsc, store=False)
        elif reps == 1:
            _emit_body(nc, tc, pools, aps, variant)
        else:
            # HW loop for timing runs: per-iteration back-edge cost ~2us.
            with tc.For_i(0, reps, 1):
                _emit_body(nc, tc, pools, aps, variant)

    nc.compile()
    _cache[key] = nc
    return nc


def run(inputs, reps=1, variant="full", n_cores=N_CORES, trace=False, **trace_kwargs):
    nc = _build(reps, variant)
    x = np.ascontiguousarray(inputs["x"], dtype=np.float32).reshape(B, C, HW)
    ctx = np.ascontiguousarray(inputs["context"], dtype=np.float32).reshape(B, C, HW)
    w1t = np.ascontiguousarray(inputs["w1"].T.astype(np.float32) * (1.0 / HW))
    b1r = np.ascontiguousarray(inputs["b1"], dtype=np.float32).reshape(MH, P, 1)
    in_maps = [
        {
            "x": x[i * B_LOC : (i + 1) * B_LOC],
            "ctx": ctx[i * B_LOC : (i + 1) * B_LOC],
            "w1t": w1t,
            "b1r": b1r,
        }
        for i in range(n_cores)
    ]
    res = run_bass_kernel_spmd(
        nc, in_maps, list(range(n_cores)), trace=trace, **trace_kwargs
    )
    out = np.concatenate([r["out"] for r in res.results], axis=0)
    if n_cores == N_CORES:
        out = out.reshape(B, C, 64, 64)
    return out, res


def kernel(**inputs):
    out, _ = run(inputs)
    return out
